# revision 1
# baseline (speedup 1.0000x reference)
"""Trainium2 Bass kernel for multi-scale deformable attention (Deformable DETR).

Sharding: data-parallel over batch — core b handles batch element b (BS=8,
n_cores=8). Per core:
  1. load query/input_flatten shard + replicated weights,
  2. offsets/attention projections (PE matmuls, fp32r) + softmax,
  3. value = input_flatten @ W_val, written to DRAM per level twice as
     even/odd "pair planes" per head (cell pairs [s, s+1] at 256B alignment);
     levels are projected high-to-low so small-level gathers start early,
  4. per-sample gather indices + bilinear/attention weights on DVE,
  5. dma_gather: 2 x 256B descriptors per (query, head, level, point)
     (one per bilinear y-row; each descriptor = the x-pair for one head),
     split into 1024-descriptor calls (ucode SWDGE ring limit),
  6. DVE combine of gathered pairs with per-cell slot weights -> samp[q,h,dh],
  7. output projection (PE, fp32r) and DMA out.
"""

import os
from contextlib import ExitStack

import numpy as np

import concourse.bass as bass
import concourse.tile as tile
from concourse import bacc, mybir
from concourse.bass_utils import run_bass_kernel_spmd
from concourse import library_config

dt = mybir.dt
Alu = mybir.AluOpType
Act = mybir.ActivationFunctionType
AxX = mybir.AxisListType.X

# ---- problem constants (hardcoded per the task spec) ----
BS = 8
NQ = 900
D = 256
NH = 8
NL = 3
NP = 4
DH = 32
SHAPES = [(128, 128), (64, 64), (32, 32)]
STARTS = [0, 16384, 20480]
SIZES = [16384, 4096, 1024]
SUM_HW = 21504
# per-level pair-plane rows (incl. pad row so odd pairs stay 256B-aligned)
NPAIRL = [SIZES[l] // 2 + 2 for l in range(NL)]  # [8194, 2050, 514]
NCH = 8                # query chunks of 128
MAGIC = 12582912.0     # 1.5 * 2**23: float32 round-to-nearest trick
KGRP = 4               # value tiles per DRAM write group
NGRP = SUM_HW // (128 * KGRP)  # 42; groups 0-31 = lvl0, 32-39 = lvl1, 40-41 = lvl2
NSLOT = NL * NP * NCH  # 96 descriptor slots per (head, yblock)
NIDX = NSLOT * 128     # 12288 descriptors per (head, yblock)

F32R = dt.float32r

USE_F32R = os.environ.get("KF32R", "1") == "1"
PHASES = os.environ.get("KPHASES", "ABCD")
MM_DT = F32R if USE_F32R else dt.float32


def _r(ap):
    return ap.bitcast(F32R) if USE_F32R else ap


def group_level(g):
    """value-tile group -> pyramid level (token ranges align with groups)."""
    if g < 32:
        return 0
    return 1 if g < 40 else 2


def build_kernel(ctx: ExitStack, tc: tile.TileContext, io: dict):
    nc = tc.nc

    cpool = ctx.enter_context(tc.tile_pool(name="const", bufs=1))
    # ---- constants / persistent tensors ----
    ident = cpool.tile([128, 128], dt.float32)
    ones1 = cpool.tile([1, 128], dt.float32)
    w_val = cpool.tile([128, 2, D], MM_DT)
    w_out = cpool.tile([128, 2, D], MM_DT)
    b_val = cpool.tile([1, D], dt.float32)
    b_out = cpool.tile([1, D], dt.float32)
    samp = cpool.tile([128, NCH, D], dt.float32)
    uasm = cpool.tile([128, 16, NSLOT, 2], dt.float32)
    wrap = cpool.tile([128, 2, NH, NSLOT * 8], dt.int16)

    nc.gpsimd.load_library(library_config.mlp)
    nc.sync.dma_start(ident[:], io["ident"])
    nc.sync.dma_start(ones1[:], io["ones1"].unsqueeze(0))
    nc.vector.memset(samp[:], 0.0)

    wvf = cpool.tile([128, 2, D], dt.float32)
    wof = cpool.tile([128, 2, D], dt.float32)
    nc.sync.dma_start(wvf[:, 0], io["W_val"][0:128, :])
    nc.sync.dma_start(wvf[:, 1], io["W_val"][128:256, :])
    nc.sync.dma_start(wof[:, 0], io["W_out"][0:128, :])
    nc.sync.dma_start(wof[:, 1], io["W_out"][128:256, :])
    nc.vector.tensor_copy(w_val[:], wvf[:])
    nc.vector.tensor_copy(w_out[:], wof[:])
    nc.sync.dma_start(b_val[:], io["b_val"].unsqueeze(0))
    nc.sync.dma_start(b_out[:], io["b_out"].unsqueeze(0))

    # pools: vpool (phase B) sits below apool in the stack so closing the
    # phase-A pools frees space for the gather pools without serializing B.
    bctx = ExitStack()
    vpool = bctx.enter_context(tc.tile_pool(name="valpool", bufs=2))
    psb = bctx.enter_context(tc.tile_pool(name="psumB", bufs=2, space="PSUM"))
    # =================== PHASE A: offsets / attn / weights ===================
    with tc.tile_pool(name="phaseA", bufs=1) as apool, \
         tc.tile_pool(name="psumA", bufs=1, space="PSUM") as pspool, \
         tc.tile_pool(name="tmps", bufs=1) as tpool:
        qsb = apool.tile([128, NCH, D], dt.float32)
        qT = apool.tile([128, 2 * NCH, 128], MM_DT)
        w_off = apool.tile([128, 2, 192], MM_DT)
        w_attn = apool.tile([128, 2, 96], MM_DT)
        w_off_f = apool.tile([128, 2, 192], dt.float32, name="w_off_f")[:]
        w_attn_f = apool.tile([128, 2, 96], dt.float32, name="w_attn_f")[:]
        b_off = apool.tile([1, 192], dt.float32)
        b_attn = apool.tile([1, 96], dt.float32)
        off = apool.tile([128, NCH, 192], dt.float32)
        attn = apool.tile([128, NCH, 96], dt.float32)
        ref = apool.tile([128, NCH, 6], dt.float32)

        nc.sync.dma_start(w_off_f[:, 0], io["W_off"][0:128, :])
        nc.sync.dma_start(w_off_f[:, 1], io["W_off"][128:256, :])
        nc.sync.dma_start(w_attn_f[:, 0], io["W_attn"][0:128, :])
        nc.sync.dma_start(w_attn_f[:, 1], io["W_attn"][128:256, :])
        nc.vector.tensor_copy(w_off[:], w_off_f[:])
        nc.vector.tensor_copy(w_attn[:], w_attn_f[:])
        nc.sync.dma_start(b_off[:], io["b_off"].unsqueeze(0))
        nc.sync.dma_start(b_attn[:], io["b_attn"].unsqueeze(0))

        # query & reference points; chunk 7 has only 4 valid rows -> zero first
        nc.vector.memset(qsb[:, 7], 0.0)
        nc.vector.memset(ref[:, 7], 0.0)
        for c in range(NCH):
            rows = 128 if c < 7 else NQ - 7 * 128
            nc.sync.dma_start(qsb[0:rows, c], io["query"][c * 128:c * 128 + rows, :])
            nc.sync.dma_start(ref[0:rows, c], io["ref"][c * 128:c * 128 + rows, :])

        # transpose query chunks: qT[:, c*2+k] = query_chunk_c[:, k*128:...].T
        for c in range(NCH):
            for k in range(2):
                pst = pspool.tile([128, 128], dt.float32, tag="pst")
                nc.tensor.transpose(pst[:], qsb[:, c, k * 128:(k + 1) * 128], ident[:])
                nc.scalar.copy(qT[:, c * 2 + k], pst[:])

        # offsets / attn projections
        for c in range(NCH):
            pso = pspool.tile([128, 192], dt.float32, tag="pso")
            nc.tensor.matmul(pso[:], lhsT=ones1[:], rhs=b_off[:], start=True, stop=False)
            nc.tensor.matmul(pso[:], lhsT=qT[:, c * 2], rhs=w_off[:, 0],
                             start=False, stop=False)
            nc.tensor.matmul(pso[:], lhsT=qT[:, c * 2 + 1], rhs=w_off[:, 1],
                             start=False, stop=True)
            nc.scalar.copy(off[:, c], pso[:])

            psa = pspool.tile([128, 96], dt.float32, tag="psa")
            nc.tensor.matmul(psa[:], lhsT=ones1[:], rhs=b_attn[:], start=True, stop=False)
            nc.tensor.matmul(psa[:], lhsT=qT[:, c * 2], rhs=w_attn[:, 0],
                             start=False, stop=False)
            nc.tensor.matmul(psa[:], lhsT=qT[:, c * 2 + 1], rhs=w_attn[:, 1],
                             start=False, stop=True)
            nc.scalar.copy(attn[:, c], psa[:])

        # softmax over the 12 (level, point) entries per (q, head)
        att3 = attn[:].rearrange("p c (h l) -> p c h l", h=NH, l=12)
        red = apool.tile([128, NCH, NH], dt.float32)
        nc.vector.tensor_reduce(red[:], att3, AxX, Alu.max)
        red_b = red[:].unsqueeze(-1).broadcast_to([128, NCH, NH, 12])
        nc.vector.tensor_tensor(att3, att3, red_b, op=Alu.subtract)
        nc.scalar.activation(attn[:], attn[:], Act.Exp)
        nc.vector.tensor_reduce(red[:], att3, AxX, Alu.add)
        nc.vector.reciprocal(red[:], red[:])
        nc.vector.tensor_tensor(att3, att3, red_b, op=Alu.mult)

        # ---- per-level index & weight computation ----
        ulvl = [[apool.tile([128, NCH, NH, NP, 2], dt.float32, tag=f"u{l}{yb}",
                            name=f"u{l}{yb}")
                 for yb in range(2)] for l in range(NL)]
        idxf = [[apool.tile([128, NCH, NH, NP], dt.float32, tag=f"i{l}{yb}",
                            name=f"i{l}{yb}")
                 for yb in range(2)] for l in range(NL)]

        def T(tag):
            return tpool.tile([128, NCH, NH, NP], dt.float32, tag=tag, name=tag)[:]

        def axis_interp(pfx, pcoord, E):
            """floor/frac/validity for one axis. pcoord: [128,NCH,NH,NP]."""
            n = T(pfx + "n")
            nc.vector.tensor_scalar(n, pcoord, MAGIC, None, op0=Alu.add)
            nc.vector.tensor_scalar(n, n, -MAGIC, None, op0=Alu.add)
            g = T(pfx + "g")
            nc.vector.tensor_tensor(g, n, pcoord, op=Alu.is_gt)
            x0 = T(pfx + "x0")
            nc.vector.tensor_tensor(x0, n, g, op=Alu.subtract)
            lx = T(pfx + "lx")
            nc.vector.tensor_tensor(lx, pcoord, x0, op=Alu.subtract)
            wx0 = T(pfx + "w0")
            nc.vector.tensor_scalar(wx0, lx, -1.0, 1.0, op0=Alu.mult, op1=Alu.add)
            c1 = T(pfx + "c1")
            nc.vector.tensor_scalar(c1, x0, 0.0, None, op0=Alu.is_ge)
            c2 = T(pfx + "c2")
            nc.vector.tensor_scalar(c2, x0, float(E - 1), None, op0=Alu.is_le)
            v0 = T(pfx + "v0")
            nc.vector.tensor_tensor(v0, c1, c2, op=Alu.mult)
            x1 = T(pfx + "x1")
            nc.vector.tensor_scalar(x1, x0, 1.0, None, op0=Alu.add)
            nc.vector.tensor_scalar(c1, x1, 0.0, None, op0=Alu.is_ge)
            nc.vector.tensor_scalar(c2, x1, float(E - 1), None, op0=Alu.is_le)
            v1 = T(pfx + "v1")
            nc.vector.tensor_tensor(v1, c1, c2, op=Alu.mult)
            return x0, x1, lx, wx0, v0, v1

        off6 = off[:].rearrange("p c (h l2 q x) -> p c h l2 q x",
                                h=NH, l2=NL, q=NP, x=2)
        att5 = attn[:].rearrange("p c (h l2 q) -> p c h l2 q", h=NH, l2=NL)

        for l in range(NL):
            Hl, Wl = SHAPES[l]
            # px = ref_x * W - 0.5 + off_x  (offsets/norm*W cancels)
            rwx = apool.tile([128, NCH], dt.float32, tag="rwx", name="rwx")[:]
            rwy = apool.tile([128, NCH], dt.float32, tag="rwy", name="rwy")[:]
            nc.vector.tensor_scalar(rwx, ref[:, :, 2 * l], float(Wl), -0.5,
                                    op0=Alu.mult, op1=Alu.add)
            nc.vector.tensor_scalar(rwy, ref[:, :, 2 * l + 1], float(Hl), -0.5,
                                    op0=Alu.mult, op1=Alu.add)
            rwx_b = rwx.unsqueeze(-1).unsqueeze(-1).broadcast_to([128, NCH, NH, NP])
            rwy_b = rwy.unsqueeze(-1).unsqueeze(-1).broadcast_to([128, NCH, NH, NP])
            px = T("px")
            nc.vector.tensor_tensor(px, off6[:, :, :, l, :, 0], rwx_b, op=Alu.add)
            py = T("py")
            nc.vector.tensor_tensor(py, off6[:, :, :, l, :, 1], rwy_b, op=Alu.add)

            x0, x1, lx, wx0, vx0, vx1 = axis_interp("x", px, Wl)
            y0, y1, ly, wy0, vy0, vy1 = axis_interp("y", py, Hl)

            # x slot geometry
            sx = T("sx")
            nc.vector.tensor_scalar(sx, x0, 0.0, float(Wl - 2), op0=Alu.max, op1=Alu.min)
            x0c = T("x0c")
            nc.vector.tensor_scalar(x0c, x0, 0.0, float(Wl - 1), op0=Alu.max, op1=Alu.min)
            x1c = T("x1c")
            nc.vector.tensor_scalar(x1c, x1, 0.0, float(Wl - 1), op0=Alu.max, op1=Alu.min)
            d0 = T("d0")
            nc.vector.tensor_tensor(d0, x0c, sx, op=Alu.subtract)
            d1 = T("d1")
            nc.vector.tensor_tensor(d1, x1c, sx, op=Alu.subtract)
            a0 = T("a0")
            nc.vector.tensor_tensor(a0, wx0, vx0, op=Alu.mult)
            a1 = T("a1")
            nc.vector.tensor_tensor(a1, lx, vx1, op=Alu.mult)
            m0 = T("m0")
            nc.vector.tensor_tensor(m0, a0, d0, op=Alu.mult)
            m1 = T("m1")
            nc.vector.tensor_tensor(m1, a1, d1, op=Alu.mult)
            u1 = T("u1")
            nc.vector.tensor_tensor(u1, m0, m1, op=Alu.add)
            u0 = T("u0")
            nc.vector.tensor_tensor(u0, a0, a1, op=Alu.add)
            nc.vector.tensor_tensor(u0, u0, u1, op=Alu.subtract)

            # y weights (attention folded in)
            at = att5[:, :, :, l, :]
            b0 = T("b0")
            nc.vector.tensor_tensor(b0, wy0, vy0, op=Alu.mult)
            nc.vector.tensor_tensor(b0, b0, at, op=Alu.mult)
            b1 = T("b1")
            nc.vector.tensor_tensor(b1, ly, vy1, op=Alu.mult)
            nc.vector.tensor_tensor(b1, b1, at, op=Alu.mult)

            y0c = T("y0c")
            nc.vector.tensor_scalar(y0c, y0, 0.0, float(Hl - 1), op0=Alu.max, op1=Alu.min)
            y1c = T("y1c")
            nc.vector.tensor_scalar(y1c, y1, 0.0, float(Hl - 1), op0=Alu.max, op1=Alu.min)

            for yb, (yc, bw) in enumerate([(y0c, b0), (y1c, b1)]):
                uv = ulvl[l][yb]
                nc.vector.tensor_tensor(uv[:, :, :, :, 0], u0, bw, op=Alu.mult)
                nc.vector.tensor_tensor(uv[:, :, :, :, 1], u1, bw, op=Alu.mult)
                # local s = yc*W + sx ; fh = floor(s/2) = round(s/2 - 0.25);
                # parity = s - 2*fh ; idx = fh + parity * (NPAIRL[l] + 1)
                s = T("s" + str(yb))
                nc.vector.tensor_scalar(s, yc, float(Wl), None, op0=Alu.mult)
                nc.vector.tensor_tensor(s, s, sx, op=Alu.add)
                fh = T("fh" + str(yb))
                nc.vector.tensor_scalar(fh, s, 0.5, -0.25, op0=Alu.mult, op1=Alu.add)
                nc.vector.tensor_scalar(fh, fh, MAGIC, None, op0=Alu.add)
                nc.vector.tensor_scalar(fh, fh, -MAGIC, None, op0=Alu.add)
                par = T("pa" + str(yb))
                nc.vector.tensor_scalar(par, fh, -2.0, None, op0=Alu.mult)
                nc.vector.tensor_tensor(par, par, s, op=Alu.add)
                iv = idxf[l][yb][:]
                nc.vector.tensor_scalar(iv, par, float(NPAIRL[l] + 1), None,
                                        op0=Alu.mult)
                nc.vector.tensor_tensor(iv, iv, fh, op=Alu.add)

        # ---- assemble U into per-(head, yblock) slot-major layout ----
        for h in range(NH):
            for yb in range(2):
                for l in range(NL):
                    src = ulvl[l][yb][:, :, h]          # [128, NCH, NP, 2]
                    dst = uasm[:, h * 2 + yb, l * 32:(l + 1) * 32, :]
                    dst4 = dst.rearrange("p (q c) x -> p q c x", q=NP, c=NCH)
                    nc.vector.tensor_copy(dst4, src.rearrange("p c q x -> p q c x"))

        # ---- assemble indices: [128, yb, h, slot] fp32 -> int16 wrapped ----
        idxm = apool.tile([128, 2, NH, NSLOT], dt.float32)
        for yb in range(2):
            for l in range(NL):
                src = idxf[l][yb][:]                     # [128, NCH, NH, NP]
                dst = idxm[:, yb, :, l * 32:(l + 1) * 32]
                dst3 = dst.rearrange("p h (q c) -> p h q c", q=NP, c=NCH)
                nc.vector.tensor_copy(dst3, src.rearrange("p c h q -> p h q c"))
        idxi = apool.tile([128, 2 * NH * NSLOT], dt.int16)
        nc.vector.tensor_copy(idxi[:], idxm[:].rearrange("p a b c -> p (a b c)"))
        # fold 128 partitions -> 16 (desc i lives at partition i%16, slot i//16)
        idxi3 = idxi[:].rearrange("p (y h s) -> p y h s", y=2, h=NH)
        for j2 in range(8):
            src = idxi3[j2 * 16:(j2 + 1) * 16]           # [16, 2, NH, NSLOT]
            dst = wrap[0:16, :, :, :].rearrange(
                "p y h (s j) -> p y h s j", j=8)[:, :, :, :, j2]
            nc.sync.dma_start(dst, src)
        for g in range(1, 8):
            nc.sync.dma_start(wrap[g * 16:(g + 1) * 16], wrap[0:16])

    # =================== PHASE B: value projection + planes ===================
    # planes_l[l]: DRAM [NH * 2 * NPAIRL[l] * 64] f32 per level
    planes_l = [io[f"planes{l}"] for l in range(NL)]
    if True:
        xflat = io["input_flatten"]  # [21504, 256]
        for g in reversed(range(NGRP)):   # levels 2,1 first -> early gathers
            lvl = group_level(g)
            t0 = g * 128 * KGRP
            t0l = t0 - STARTS[lvl]
            xg = vpool.tile([128, KGRP, D], dt.float32, tag="xg", name="xg")
            src = xflat[t0:t0 + 128 * KGRP, :].rearrange(
                "(k p) f -> p k f", k=KGRP, p=128)
            nc.sync.dma_start(xg[:], src)
            vg = vpool.tile([128, KGRP, D], dt.float32, tag="vg", name="vg")
            for k in range(KGRP):
                xt0 = psb.tile([128, 128], dt.float32, tag="xt", bufs=3)
                xt1 = psb.tile([128, 128], dt.float32, tag="xt", bufs=3)
                nc.tensor.transpose(xt0[:], xg[:, k, 0:128], ident[:])
                nc.tensor.transpose(xt1[:], xg[:, k, 128:256], ident[:])
                xts = vpool.tile([128, 2, 128], MM_DT, tag="xts", name="xts")
                nc.scalar.copy(xts[:, 0], xt0[:])
                nc.scalar.copy(xts[:, 1], xt1[:])
                pv = psb.tile([128, D], dt.float32, tag="pv")
                nc.tensor.matmul(pv[:], lhsT=ones1[:], rhs=b_val[:],
                                 start=True, stop=False)
                nc.tensor.matmul(pv[:], lhsT=xts[:, 0], rhs=w_val[:, 0],
                                 start=False, stop=False)
                nc.tensor.matmul(pv[:], lhsT=xts[:, 1], rhs=w_val[:, 1],
                                 start=False, stop=True)
                nc.scalar.copy(vg[:, k], pv[:])
            # write planes: even plane token t at t*32; odd plane at (t+1)*32
            vg4 = vg[:].rearrange("p k (h d) -> p k h d", h=NH)
            npl = NPAIRL[lvl]
            for h in (range(NH) if "B" in PHASES else []):
                src_h = vg4[:, :, h, :]
                base_e = h * (2 * npl * 64)
                base_o = base_e + npl * 64
                dst_e = planes_l[lvl][base_e + t0l * 32:
                                      base_e + (t0l + 128 * KGRP) * 32].rearrange(
                    "(k p d) -> p k d", k=KGRP, p=128, d=32)
                nc.sync.dma_start(dst_e, src_h)
                dst_o = planes_l[lvl][base_o + (t0l + 1) * 32:
                                      base_o + (t0l + 1 + 128 * KGRP) * 32].rearrange(
                    "(k p d) -> p k d", k=KGRP, p=128, d=32)
                nc.sync.dma_start(dst_o, src_h)

    # =================== PHASE C: gather + combine ===================
    # per (h, yb, level): 4 x 1024-descriptor calls (ucode SWDGE ring limit),
    # then a per-level combine so small-level work overlaps level-0 projection
    with tc.tile_pool(name="gpool", bufs=1) as gpool, \
         tc.tile_pool(name="comb", bufs=1) as combp:
        qn = 0
        for h in range(NH):
            for yb in range(2):
                for l in range(NL):
                    g_t = gpool.tile([128, 32, 64], dt.float32, tag="g",
                                     bufs=4, name=f"g{h}{yb}{l}")
                    npl = NPAIRL[l]
                    in_ap = planes_l[l][h * 2 * npl * 64:
                                        (h + 1) * 2 * npl * 64]
                    in_ap = in_ap.rearrange("(r e) -> r e", e=64)
                    if "C" in PHASES:
                        for p in range(NP):
                            cc = l * NP + p
                            nc.gpsimd.dma_gather(
                                out_ap=g_t[:, p * 8:(p + 1) * 8, :],
                                in_ap=in_ap,
                                idxs_ap=wrap[:, yb, h, cc * 64:(cc + 1) * 64],
                                num_idxs=1024,
                                num_idxs_reg=1024,
                                elem_size=64,
                                queue_num=qn % 4,
                            )
                            qn += 1
                    else:
                        nc.vector.memset(g_t[:], 0.0)
                    g4 = g_t[:].rearrange("p s (c d) -> p s c d", c=2)
                    us = uasm[:, h * 2 + yb, l * 32:(l + 1) * 32, :]
                    u_b0 = us[:, :, 0].unsqueeze(-1).broadcast_to([128, 32, DH])
                    u_b1 = us[:, :, 1].unsqueeze(-1).broadcast_to([128, 32, DH])
                    p0 = combp.tile([128, 32, DH], dt.float32, tag="p0",
                                    bufs=2, name="p0")
                    p1 = combp.tile([128, 32, DH], dt.float32, tag="p1",
                                    bufs=2, name="p1")
                    nc.vector.tensor_tensor(p0[:], g4[:, :, 0, :], u_b0,
                                            op=Alu.mult)
                    nc.vector.tensor_tensor(p1[:], g4[:, :, 1, :], u_b1,
                                            op=Alu.mult)
                    nc.vector.tensor_tensor(p0[:], p0[:], p1[:], op=Alu.add)
                    # reduce over the 4 points; slot_local = p*8 + c
                    r2 = combp.tile([128, NCH, DH], dt.float32, tag="r2",
                                    bufs=2, name="r2")
                    s_r = p0[:].rearrange("p (q c) d -> p c d q", q=NP, c=NCH)
                    nc.vector.tensor_reduce(r2[:], s_r, AxX, Alu.add)
                    acc = samp[:].rearrange("p c (h d) -> p c h d",
                                            h=NH)[:, :, h, :]
                    nc.vector.tensor_tensor(acc, acc, r2[:], op=Alu.add)

    bctx.close()  # release phase-B pools (B work is done by now)

    # =================== PHASE D: output projection ===================
    with tc.tile_pool(name="outp", bufs=2) as opool, \
         tc.tile_pool(name="psumD", bufs=2, space="PSUM") as psd:
        for c in range(NCH):
            st0 = psd.tile([128, 128], dt.float32, tag="st0")
            st1 = psd.tile([128, 128], dt.float32, tag="st1")
            nc.tensor.transpose(st0[:], samp[:, c, 0:128], ident[:])
            nc.tensor.transpose(st1[:], samp[:, c, 128:256], ident[:])
            sts = opool.tile([128, 2, 128], MM_DT, tag="sts", name="sts")
            nc.scalar.copy(sts[:, 0], st0[:])
            nc.scalar.copy(sts[:, 1], st1[:])
            po = psd.tile([128, D], dt.float32, tag="po")
            nc.tensor.matmul(po[:], lhsT=ones1[:], rhs=b_out[:], start=True, stop=False)
            nc.tensor.matmul(po[:], lhsT=sts[:, 0], rhs=w_out[:, 0],
                             start=False, stop=False)
            nc.tensor.matmul(po[:], lhsT=sts[:, 1], rhs=w_out[:, 1],
                             start=False, stop=True)
            osb = opool.tile([128, D], dt.float32, tag="osb", name="osb")
            nc.scalar.copy(osb[:], po[:])
            rows = 128 if c < 7 else NQ - 7 * 128
            nc.sync.dma_start(io["out"][c * 128:c * 128 + rows, :], osb[0:rows])


def build_nc():
    nc = bacc.Bacc("TRN2", target_bir_lowering=False, debug=False, num_devices=8,
                   num_swdge_queues=4)
    io = {}
    io["query"] = nc.dram_tensor("query", [NQ, D], dt.float32,
                                 kind="ExternalInput").ap()
    io["ref"] = nc.dram_tensor("ref", [NQ, 6], dt.float32,
                               kind="ExternalInput").ap()
    io["input_flatten"] = nc.dram_tensor("input_flatten", [SUM_HW, D], dt.float32,
                                         kind="ExternalInput").ap()
    for nm, shape in [("W_off", [D, 192]), ("b_off", [192]),
                      ("W_attn", [D, 96]), ("b_attn", [96]),
                      ("W_val", [D, D]), ("b_val", [D]),
                      ("W_out", [D, D]), ("b_out", [D])]:
        io[nm] = nc.dram_tensor(nm, shape, dt.float32, kind="ExternalInput").ap()
    io["out"] = nc.dram_tensor("out", [NQ, D], dt.float32,
                               kind="ExternalOutput").ap()
    for l in range(NL):
        io[f"planes{l}"] = nc.dram_tensor(
            f"planes{l}", [NH * 2 * NPAIRL[l] * 64], dt.float32).ap()
    io["ident"] = nc.dram_tensor("ident", [128, 128], dt.float32,
                                 kind="ExternalInput").ap()
    io["ones1"] = nc.dram_tensor("ones1", [128], dt.float32,
                                 kind="ExternalInput").ap()

    with tile.TileContext(nc) as tc:
        with ExitStack() as ctx:
            build_kernel(ctx, tc, io)
    nc.compile()
    return nc


_NC = None


def _get_nc():
    global _NC
    if _NC is None:
        _NC = build_nc()
    return _NC


def make_in_maps(inputs):
    in_maps = []
    for b in range(BS):
        m = {
            "query": np.ascontiguousarray(inputs["query"][b], np.float32),
            "ref": np.ascontiguousarray(
                np.asarray(inputs["reference_points"][b]).reshape(NQ, 6),
                np.float32),
            "input_flatten": np.ascontiguousarray(
                inputs["input_flatten"][b], np.float32),
        }
        for nm in ["W_off", "b_off", "W_attn", "b_attn", "W_val", "b_val",
                   "W_out", "b_out"]:
            m[nm] = np.ascontiguousarray(inputs[nm], np.float32)
        m["ident"] = np.eye(128, dtype=np.float32)
        m["ones1"] = np.ones(128, dtype=np.float32)
        in_maps.append(m)
    return in_maps


def kernel(**inputs):
    nc = _get_nc()
    in_maps = make_in_maps(inputs)
    res = run_bass_kernel_spmd(nc, in_maps, core_ids=list(range(BS)))
    out = np.stack([res.results[i]["out"] for i in range(BS)])
    return out.astype(np.float32)

